# revision 3
# baseline (speedup 1.0000x reference)
"""Trainium2 Bass kernel: ActiveBlockPromptBasis (moe_routing).

Math (per batch image b):
  g   = gelu(W1x @ x_b + b1x  ++  W1t @ flux_b + b1t)        # [14, pix]
  z   = Mz.T @ g + bz          (fc2 of both MLPs fused with the 6x8
                                outer-sum expansion, in log space)  # [48, pix]
  wun = exp(z);  S = colsum(wun);  w = wun / S                # exact softmax
  P   = prompt_flat.T @ w                                     # [128, pix]
  out = conv3x3(P, conv_w)     (9 accumulating matmuls / 2-row PSUM bank)

Sharding: data-parallel over batch, one image per NeuronCore (8 cores).
"""

import numpy as np
from contextlib import ExitStack

import concourse.bass as bass
import concourse.tile as tile
from concourse import bacc, mybir
from concourse.bass_utils import run_bass_kernel_spmd

F32 = mybir.dt.float32
AFT = mybir.ActivationFunctionType

B, DIM, E = 8, 64, 128
NT, NB = 6, 8
NTK = NT * NB  # 48
NCORES = 8


def build_program(h=256, w=256, r_out=32, gelu_fn=AFT.Gelu):
    """Build the single-core Bass program (SPMD: same program on all cores)."""
    assert w == 256 and r_out % 8 == 0 and h % r_out == 0
    PIX = h * w
    PITCH = w + 2          # row window with 1 zero spacer col each side
    IR_MAX = r_out + 2     # input rows per strip incl halo

    nc = bacc.Bacc("TRN2", target_bir_lowering=False, debug=False,
                   enable_asserts=False)

    # --- DRAM I/O (per-core slices / replicated small weights) ---
    xf_d = nc.dram_tensor("xf", [128, PIX], F32, kind="ExternalInput")
    fb_d = nc.dram_tensor("fb", [65, PIX], F32, kind="ExternalInput")
    wa_d = nc.dram_tensor("wa", [128, 14], F32, kind="ExternalInput")
    wb_d = nc.dram_tensor("wb", [65, 14], F32, kind="ExternalInput")
    mz_d = nc.dram_tensor("mz", [14, NTK], F32, kind="ExternalInput")
    bz_d = nc.dram_tensor("bz", [NTK, 1], F32, kind="ExternalInput")
    on48_d = nc.dram_tensor("on48", [NTK, 1], F32, kind="ExternalInput")
    on1_d = nc.dram_tensor("on1", [1, NTK], F32, kind="ExternalInput")
    pt_d = nc.dram_tensor("pt", [NTK, E], F32, kind="ExternalInput")
    wt_d = nc.dram_tensor("wt", [9, E, E], F32, kind="ExternalInput")
    out_d = nc.dram_tensor("out", [E, PIX], F32, kind="ExternalOutput")

    with tile.TileContext(nc) as tc, ExitStack() as ctx:
        consts = ctx.enter_context(tc.tile_pool(name="consts", bufs=1))
        pin = ctx.enter_context(tc.tile_pool(name="pin", bufs=4))
        psb = ctx.enter_context(tc.tile_pool(name="psb", bufs=4))
        pg_pool = ctx.enter_context(tc.tile_pool(name="pg", bufs=IR_MAX // 2 + 2))
        ppool = ctx.enter_context(
            tc.tile_pool(name="ppsum", bufs=8, space="PSUM"))
        pP = ctx.enter_context(tc.tile_pool(name="pP", bufs=2))
        pout = ctx.enter_context(tc.tile_pool(name="pout", bufs=2))

        # --- load constants once ---
        wa_sb = consts.tile([128, 14], F32)
        nc.sync.dma_start(out=wa_sb[:], in_=wa_d[:])
        wb_sb = consts.tile([65, 14], F32)
        nc.sync.dma_start(out=wb_sb[:], in_=wb_d[:])
        mz_sb = consts.tile([14, NTK], F32)
        nc.sync.dma_start(out=mz_sb[:], in_=mz_d[:])
        bz_sb = consts.tile([NTK, 1], F32)
        nc.sync.dma_start(out=bz_sb[:], in_=bz_d[:])
        on48_sb = consts.tile([NTK, 1], F32)
        nc.sync.dma_start(out=on48_sb[:], in_=on48_d[:])
        on1_sb = consts.tile([1, NTK], F32)
        nc.sync.dma_start(out=on1_sb[:], in_=on1_d[:])
        pt_sb = consts.tile([NTK, E], F32)
        nc.sync.dma_start(out=pt_sb[:], in_=pt_d[:])
        wt_sb = consts.tile([E, 9 * E], F32)
        for t in range(9):
            nc.sync.dma_start(out=wt_sb[:, t * E:(t + 1) * E], in_=wt_d[t])

        last_exp_inst = None
        n_strips = h // r_out

        for s in range(n_strips):
            y0, y1 = s * r_out, (s + 1) * r_out
            r0, r1 = max(0, y0 - 1), min(h - 1, y1)  # input rows incl halo
            ir = r1 - r0 + 1

            P_t = pP.tile([128, IR_MAX * PITCH], F32, tag="P")
            P3 = P_t[:].rearrange("p (r c) -> p r c", c=PITCH)
            # zero the spacer columns (left/right zero padding for the conv)
            nc.vector.memset(P3[:, :ir, 0:1], 0.0)
            nc.vector.memset(P3[:, :ir, PITCH - 1:PITCH], 0.0)

            chunks = []
            r = r0
            while r <= r1:
                nrows = min(2, r1 - r + 1)
                chunks.append((r, nrows))
                r += nrows

            # ---- stage 1 phase A: fc1 + gelu (one ACT table set) ----
            g_tiles = []
            first_gelu_inst = None
            for (r, nrows) in chunks:
                npix = nrows * w
                off = r * w
                xf_t = pin.tile([128, 512], F32, tag="xf")
                nc.sync.dma_start(out=xf_t[:, :npix], in_=xf_d[:, off:off + npix])
                fb_t = pin.tile([65, 512], F32, tag="fb")
                nc.sync.dma_start(out=fb_t[:, :npix], in_=fb_d[:, off:off + npix])
                pg = ppool.tile([14, 512], F32, tag="bank")
                nc.tensor.matmul(pg[:, :npix], wa_sb[:], xf_t[:, :npix],
                                 start=True, stop=False)
                nc.tensor.matmul(pg[:, :npix], wb_sb[:], fb_t[:, :npix],
                                 start=False, stop=True)
                g_t = pg_pool.tile([14, 512], F32, tag="g")
                inst = nc.scalar.activation(g_t[:, :npix], pg[:, :npix], gelu_fn)
                if first_gelu_inst is None:
                    first_gelu_inst = inst
                g_tiles.append(g_t)

            # keep ACT-engine phases ordered across strips so walrus doesn't
            # re-load activation tables on interleaved gelu/exp runs
            if last_exp_inst is not None and first_gelu_inst is not None:
                bass._add_dep_helper(first_gelu_inst.ins, last_exp_inst.ins,
                                     sync=True, reason="act-table-phase-order")

            # ---- stage 1 phase B: fc2+expand, softmax, prompt matmul ----
            for ci, (r, nrows) in enumerate(chunks):
                npix = nrows * w
                g_t = g_tiles[ci]
                pz = ppool.tile([NTK, 512], F32, tag="bank")
                nc.tensor.matmul(pz[:, :npix], mz_sb[:], g_t[:, :npix])
                wun = psb.tile([NTK, 512], F32, tag="wun")
                inst = nc.scalar.activation(wun[:, :npix], pz[:, :npix],
                                            AFT.Exp, bias=bz_sb[:])
                last_exp_inst = inst
                ps = ppool.tile([1, 512], F32, tag="bank")
                nc.tensor.matmul(ps[:, :npix], on48_sb[:], wun[:, :npix])
                rr = psb.tile([1, 512], F32, tag="rr")
                nc.vector.reciprocal_approx_fast(rr[:, :npix], ps[:, :npix])
                prb = ppool.tile([NTK, 512], F32, tag="bank")
                nc.tensor.matmul(prb[:, :npix], on1_sb[:], rr[:, :npix])
                w_t = psb.tile([NTK, 512], F32, tag="w")
                nc.vector.tensor_mul(w_t[:, :npix], wun[:, :npix], prb[:, :npix])
                pp = ppool.tile([128, 512], F32, tag="bank")
                nc.tensor.matmul(pp[:, :npix], pt_sb[:], w_t[:, :npix])
                lr = r - r0
                nc.vector.tensor_copy(
                    P3[:, lr:lr + nrows, 1:1 + w],
                    pp[:, :npix].rearrange("p (r c) -> p r c", c=w))

            # ---- conv3x3 over this strip ----
            drain_toggle = 0
            for g0 in range(y0, y1, 8):
                out_sb = pout.tile([128, 2048], F32, tag="outsb")
                for p in range(4):
                    yA = g0 + 2 * p
                    pcv = ppool.tile([128, 512], F32, tag="bank")
                    taps = []
                    for ky in (1, 0, 2):
                        rlo, rhi = yA, yA + 1
                        if yA + ky - 1 < 0:
                            rlo = yA + 1
                        if yA + 1 + ky - 1 > h - 1:
                            rhi = yA
                        for kx in (0, 1, 2):
                            taps.append((ky, kx, rlo, rhi))
                    for ti, (ky, kx, rlo, rhi) in enumerate(taps):
                        nr = rhi - rlo + 1
                        lr = rlo + ky - 1 - r0
                        tap = ky * 3 + kx
                        nc.tensor.matmul(
                            pcv[:, (rlo - yA) * w:(rhi - yA + 1) * w],
                            wt_sb[:, tap * E:(tap + 1) * E],
                            P3[:, lr:lr + nr, kx:kx + w],
                            start=(ti == 0), stop=(ti == len(taps) - 1))
                    dst = out_sb[:, p * 512:(p + 1) * 512]
                    if drain_toggle % 2 == 0:
                        nc.vector.tensor_copy(dst, pcv[:])
                    else:
                        nc.scalar.copy(dst, pcv[:])
                    drain_toggle += 1
                nc.sync.dma_start(out=out_d[:, g0 * w:(g0 + 8) * w],
                                  in_=out_sb[:])

    nc.compile()
    return nc


_cache = {}


def get_program(h=256, w=256, r_out=32, gelu_fn=AFT.Gelu):
    key = (h, w, r_out, gelu_fn)
    if key not in _cache:
        _cache[key] = build_program(h, w, r_out, gelu_fn)
    return _cache[key]


def make_weight_inputs(prompt, conv_w, b_fc1_w, b_fc1_b, b_fc2_w, b_fc2_b,
                       t_fc1_w, t_fc1_b, t_fc2_w, t_fc2_b):
    f = np.float32
    wa = np.zeros((128, 14), f)
    wa[:64, :8] = b_fc1_w.T
    wa[64:128, 8:14] = t_fc1_w[:, :64].T
    wb = np.zeros((65, 14), f)
    wb[:64, 8:14] = t_fc1_w[:, 64:].T
    wb[64, :8] = b_fc1_b
    wb[64, 8:14] = t_fc1_b
    mz = np.zeros((14, NTK), f)
    bz = np.zeros((NTK, 1), f)
    for t in range(NT):
        for k in range(NB):
            c = t * NB + k
            mz[:8, c] = b_fc2_w[k, :]
            mz[8:, c] = t_fc2_w[t, :]
            bz[c, 0] = b_fc2_b[k] + t_fc2_b[t]
    return {
        "wa": wa,
        "wb": wb,
        "mz": mz,
        "bz": bz,
        "on48": np.ones((NTK, 1), f),
        "on1": np.ones((1, NTK), f),
        "pt": np.ascontiguousarray(prompt.reshape(NTK, E).astype(f)),
        "wt": np.ascontiguousarray(
            conv_w.transpose(2, 3, 1, 0).reshape(9, E, E).astype(f)),
    }


def make_core_inputs(x_b, flux_b, weights, h, w):
    PIX = h * w
    f = np.float32
    xf = np.concatenate(
        [x_b.reshape(DIM, PIX), flux_b[:64].reshape(64, PIX)], axis=0)
    fb = np.concatenate(
        [flux_b[64:].reshape(64, PIX), np.ones((1, PIX), f)], axis=0)
    m = {"xf": np.ascontiguousarray(xf, dtype=f),
         "fb": np.ascontiguousarray(fb, dtype=f)}
    m.update(weights)
    return m


def kernel(x, flux, prompt, conv_w, b_fc1_w, b_fc1_b, b_fc2_w, b_fc2_b,
           t_fc1_w, t_fc1_b, t_fc2_w, t_fc2_b):
    x = np.asarray(x, np.float32)
    flux = np.asarray(flux, np.float32)
    flux = np.where(np.isnan(flux), np.float32(0), flux)
    h, w = x.shape[2], x.shape[3]

    nc = get_program(h=h, w=w)
    weights = make_weight_inputs(
        np.asarray(prompt, np.float32), np.asarray(conv_w, np.float32),
        np.asarray(b_fc1_w, np.float32), np.asarray(b_fc1_b, np.float32),
        np.asarray(b_fc2_w, np.float32), np.asarray(b_fc2_b, np.float32),
        np.asarray(t_fc1_w, np.float32), np.asarray(t_fc1_b, np.float32),
        np.asarray(t_fc2_w, np.float32), np.asarray(t_fc2_b, np.float32))
    in_maps = [make_core_inputs(x[i], flux[i], weights, h, w)
               for i in range(NCORES)]
    res = run_bass_kernel_spmd(nc, in_maps, list(range(NCORES)))
    out = np.stack([res.results[i]["out"].reshape(E, h, w)
                    for i in range(NCORES)], axis=0)
    return out
